# revision 9
# baseline (speedup 1.0000x reference)
"""Trainium2 Bass kernel for causal self-attention (B=2, T=2048, C=1024, H=16).

Sharding: tensor-parallel over heads. Each of the 8 cores owns 2 heads:
  - qkv weight column slices (128 q cols, 128 k cols, 128 v cols),
  - the full attention for its (batch, head) pairs,
  - a row-slice (128 rows) of w_proj -> partial [B*T, C] output.
Host side: x is transposed/cast to bf16 once (shared by all cores); the 8
partial proj outputs are summed + b_proj to form the final output.

Per-core device kernel (all matmuls bf16, fp32 accumulate):
  1. qkv^T projection: Q^T,K^T produced with head-dim on partitions
     ([128 = 2 heads x 64, T]); V produced in natural [t, v] layout with an
     appended ones column so the softmax denominator falls out of the AV
     matmul as a 65th output row.
  2. Causal attention over 128-wide k chunks x 512-wide q tiles computing
     S^T = [k, q] blocks (PE), additive mask on diagonal chunks (DVE),
     exp(scale*s) PSUM->SBUF bf16 (ACT), AV accumulation (PE).
  3. Normalize: reciprocal of the denominator row (DVE), partition-broadcast
     (GPSIMD), multiply (DVE) -> attn_out^T bf16.
  4. proj: attn_out^T chunks as stationary against w_proj rows -> partial
     fp32 [B*T, C] DMA'd out.
"""

import numpy as np
import ml_dtypes
from contextlib import ExitStack

import concourse.bass as bass
import concourse.bacc as bacc
import concourse.mybir as mybir
import concourse.tile as tile

BF16 = mybir.dt.bfloat16
F32 = mybir.dt.float32

B, T, C = 2, 2048, 1024
H = 16
D = C // H  # 64
SCALE = D ** -0.5
N_CORES = 8
HEADS_PER_CORE = H // N_CORES  # 2
CL = HEADS_PER_CORE * D  # 128 local channels
P = 128
QT = 512  # q tile width (free dim of S^T blocks)
KC = 128  # k chunk (partition dim of S^T blocks)
MASK_VAL = -1.0e5  # pre-scale additive mask; exp(scale*(-1e5)) == 0 in fp32


def build_nc(t_batch=T, n_batch=B):
    """Build the per-core Bass module. t_batch/n_batch shrinkable for sim."""
    tt = t_batch * n_batch
    nqt = t_batch // QT  # q tiles per batch
    ntt = tt // QT       # t tiles for the qkv projection phase
    ntc = tt // P        # 128-wide t chunks
    cdim = C
    nco = cdim // P      # contraction chunks for qkv projection

    nc = bacc.Bacc("TRN2", target_bir_lowering=False)
    xt = nc.dram_tensor("xt", [cdim, tt], BF16, kind="ExternalInput")
    wq = nc.dram_tensor("wq", [cdim, 3 * CL], BF16, kind="ExternalInput")
    bq = nc.dram_tensor("bq", [3 * CL], F32, kind="ExternalInput")
    bv = nc.dram_tensor("bv", [P, CL], F32, kind="ExternalInput")
    wp = nc.dram_tensor("wp", [CL, C], BF16, kind="ExternalInput")
    mk = nc.dram_tensor("mk", [4 * P, QT], F32, kind="ExternalInput")
    op = nc.dram_tensor("op", [tt, C], F32, kind="ExternalOutput")

    with tile.TileContext(nc) as tc, ExitStack() as ctx:
        singles = ctx.enter_context(tc.tile_pool(name="singles", bufs=1))

        qT = singles.tile([P, tt], BF16)   # rows: head h in [64h, 64h+64)
        kT = singles.tile([P, tt], BF16)
        vsb = singles.tile([P, ntc, 2 * (D + 1)], BF16)  # V_ext, both heads
        aoT = singles.tile([P, tt], BF16)  # normalized attn out^T
        wq_sb = singles.tile([P, nco, 3 * CL], BF16)
        bq_sb = singles.tile([P, 3], F32)
        bv_sb = singles.tile([P, CL], F32)
        wp_sb = singles.tile([CL, C], BF16)
        mask_sb = singles.tile([P, 4, QT], F32)

        # Preloads go through the single SWDGE queue so downstream consumers
        # only inherit one DMA-semaphore wait (walrus caps waits per inst).
        nc.gpsimd.dma_start(wq_sb, wq.rearrange("(co p) m -> p co m", p=P))
        nc.gpsimd.dma_start(bq_sb, bq.rearrange("(m p) -> p m", p=P))
        nc.gpsimd.dma_start(bv_sb, bv[:, :])
        nc.gpsimd.dma_start(wp_sb, wp[:, :])
        nc.gpsimd.dma_start(mask_sb, mk.rearrange("(mm p) q -> p mm q", p=P))

        # ---- Phase 1: qkv projection ----
        with tc.tile_pool(name="xt_pool", bufs=2) as xt_pool, \
             tc.tile_pool(name="qkv_ps", bufs=2, space="PSUM") as qkv_ps:
            xt_r = xt.rearrange("(co p) t -> p co t", p=P)
            for j in range(ntt):
                ts = slice(j * QT, (j + 1) * QT)
                xt_t = xt_pool.tile([P, nco, QT], BF16, tag="xt")
                nc.sync.dma_start(xt_t, xt_r[:, :, ts])
                # Q^T and K^T: W chunk stationary, x^T moving
                for mi, dst in ((0, qT), (1, kT)):
                    ps = qkv_ps.tile([P, QT], F32, tag="qk")
                    for co in range(nco):
                        nc.tensor.matmul(
                            ps,
                            lhsT=wq_sb[:, co, mi * CL:(mi + 1) * CL],
                            rhs=xt_t[:, co, :],
                            start=(co == 0), stop=(co == nco - 1),
                        )
                    nc.vector.tensor_tensor(
                        dst[:, ts], ps,
                        bq_sb[:, mi:mi + 1].to_broadcast((P, QT)),
                        op=mybir.AluOpType.add)
                # V natural: x^T chunk stationary, W_v moving
                for t2 in range(QT // P):
                    tg = j * (QT // P) + t2
                    psv = qkv_ps.tile([P, CL], F32, tag="v")
                    for co in range(nco):
                        nc.tensor.matmul(
                            psv,
                            lhsT=xt_t[:, co, t2 * P:(t2 + 1) * P],
                            rhs=wq_sb[:, co, 2 * CL:3 * CL],
                            start=(co == 0), stop=(co == nco - 1),
                        )
                    for h in range(2):
                        o0 = h * (D + 1)
                        nc.vector.tensor_add(
                            vsb[:, tg, o0:o0 + D],
                            psv[:, h * D:(h + 1) * D], bv_sb[:, h * D:(h + 1) * D])
                        nc.vector.memset(vsb[:, tg, o0 + D:o0 + D + 1], 1.0)

        # ---- Phase 2: causal attention ----
        with tc.tile_pool(name="s_ps", bufs=2, space="PSUM") as s_ps, \
             tc.tile_pool(name="o_ps", bufs=2, space="PSUM") as o_ps, \
             tc.tile_pool(name="pt_pool", bufs=3) as pt_pool, \
             tc.tile_pool(name="nrm_dram", bufs=2, space="DRAM") as nrm_dram, \
             tc.tile_pool(name="nrm_pool", bufs=2) as nrm_pool:
            for b in range(n_batch):
                base = b * t_batch
                for j in range(nqt):
                    qs = slice(base + j * QT, base + (j + 1) * QT)
                    os_ = [o_ps.tile([D + 1, QT], F32, tag=f"o{h}", name=f"o{h}") for h in range(2)]
                    nch = (j + 1) * (QT // KC)
                    for m in range(nch):
                        ks = slice(base + m * KC, base + m * KC + KC)
                        mm = m - j * (QT // KC)
                        pts = []
                        for h in range(2):
                            hp = slice(64 * h, 64 * h + 64)
                            s = s_ps.tile([P, QT], F32, tag=f"s{h}")
                            nc.tensor.matmul(s, lhsT=kT[hp, ks], rhs=qT[hp, qs],
                                             start=True, stop=True)
                            if mm >= 0:
                                nc.vector.tensor_add(s, s, mask_sb[:, mm, :])
                            pt = pt_pool.tile([P, QT], BF16, tag=f"pt{h}")
                            nc.scalar.activation(
                                pt, s, mybir.ActivationFunctionType.Exp,
                                bias=0.0, scale=SCALE)
                            pts.append(pt)
                        tg = (base + m * KC) // P
                        for h in range(2):
                            nc.tensor.matmul(
                                os_[h],
                                lhsT=vsb[:, tg, h * (D + 1):(h + 1) * (D + 1)],
                                rhs=pts[h],
                                start=(m == 0), stop=(m == nch - 1),
                            )
                    for h in range(2):
                        rec = nrm_pool.tile([1, QT], F32, tag="rec")
                        nc.vector.reciprocal(rec, os_[h][D:D + 1, :])
                        dr = nrm_dram.tile([QT], F32, tag="dr")
                        nc.sync.dma_start(dr, rec[0:1, :])
                        bc = nrm_pool.tile([D, QT], F32, tag="bc")
                        nc.sync.dma_start(
                            bc, bass.AP(dr.tensor, dr.offset, [[0, D]] + list(dr.ap)))
                        nc.vector.tensor_mul(aoT[64 * h:64 * h + 64, qs],
                                             os_[h][0:D, :], bc)

        # ---- Phase 3: output projection (partial) ----
        with tc.tile_pool(name="p_ps", bufs=3, space="PSUM") as p_ps, \
             tc.tile_pool(name="p_sb", bufs=3) as p_sb:
            for t2 in range(ntc):
                lhs = aoT[:, t2 * P:(t2 + 1) * P]
                for n in range(C // QT):
                    pp = p_ps.tile([P, QT], F32, tag="pp")
                    nc.tensor.matmul(pp, lhsT=lhs, rhs=wp_sb[:, n * QT:(n + 1) * QT],
                                     start=True, stop=True)
                    ob = p_sb.tile([P, QT], F32, tag="ob")
                    nc.any.tensor_copy(ob, pp)
                    nc.sync.dma_start(
                        op[t2 * P:(t2 + 1) * P, n * QT:(n + 1) * QT], ob)

    nc.finalize()
    return nc


def make_masks():
    kl = np.arange(P)[:, None]
    ql = np.arange(QT)[None, :]
    out = np.zeros((4 * P, QT), np.float32)
    for mm in range(4):
        out[mm * P:(mm + 1) * P] = np.where(mm * P + kl > ql, MASK_VAL, 0.0)
    return out


def make_in_maps(x, w_qkv, b_qkv, w_proj, t_batch=T, n_batch=B):
    bf = ml_dtypes.bfloat16
    tt = t_batch * n_batch
    x2 = np.ascontiguousarray(x.reshape(tt, C))
    xt = np.ascontiguousarray(x2.T).astype(bf)
    masks = make_masks()
    in_maps = []
    for i in range(N_CORES):
        cs = slice(CL * i, CL * (i + 1))
        wq_c = np.concatenate(
            [w_qkv[:, cs], w_qkv[:, C + CL * i:C + CL * (i + 1)],
             w_qkv[:, 2 * C + CL * i:2 * C + CL * (i + 1)]], axis=1).astype(bf)
        bq_c = np.concatenate(
            [b_qkv[cs], b_qkv[C + CL * i:C + CL * (i + 1)],
             b_qkv[2 * C + CL * i:2 * C + CL * (i + 1)]]).astype(np.float32)
        bv_c = np.ascontiguousarray(np.broadcast_to(
            b_qkv[2 * C + CL * i:2 * C + CL * (i + 1)][None, :],
            (P, CL))).astype(np.float32)
        wp_c = np.ascontiguousarray(w_proj[cs, :]).astype(bf)
        in_maps.append({
            "xt": xt, "wq": np.ascontiguousarray(wq_c), "bq": bq_c,
            "bv": bv_c, "wp": wp_c, "mk": masks,
        })
    return in_maps


_CACHE = {}


def kernel(x, w_qkv, b_qkv, w_proj, b_proj):
    from concourse.bass_utils import run_bass_kernel_spmd

    x = np.asarray(x, np.float32)
    w_qkv = np.asarray(w_qkv, np.float32)
    b_qkv = np.asarray(b_qkv, np.float32)
    w_proj = np.asarray(w_proj, np.float32)
    b_proj = np.asarray(b_proj, np.float32)

    if "nc" not in _CACHE:
        _CACHE["nc"] = build_nc()
    nc = _CACHE["nc"]
    in_maps = make_in_maps(x, w_qkv, b_qkv, w_proj)
    res = run_bass_kernel_spmd(nc, in_maps, core_ids=list(range(N_CORES)))
    partial = np.zeros((B * T, C), np.float64)
    for r in res.results:
        partial += r["op"].astype(np.float64)
    out = (partial + b_proj.astype(np.float64)).astype(np.float32)
    return out.reshape(B, T, C)
